# revision 4
# baseline (speedup 1.0000x reference)
"""Multi-head causal attention on 8 Trainium2 NeuronCores.

Sharding: core = (batch b in 0..3) x (head-group g in 0..1, 4 heads each).
Each core computes a partial output (its head-group's contribution through
wp); host sums the two partials per batch.

Per-core kernel (fp32r matmuls, fp32 accumulation):
  xT        = transpose(x[b])                      (PE transpose)
  qT/kT     = w^T @ xT   per head-pair (2 heads packed on 128 partitions)
  v         = x @ wv     all 4 heads, natural [t, hs] layout (+ ones column)
  S^T tiles = K Q^T      [tk=128, tq<=512] per head, causal-trimmed
  P^T       = exp(S^T / 8)   (ACT, batched over 4 psum banks)
  y'^T,l    = V'^T @ P^T     accumulated in psum ([65, 512]; row 64 = denom)
  y^T       = y'^T * (1/l)   (recip + partition-broadcast + mul)
  out       = y^T.T @ wp_g   -> DRAM partial
"""

import sys
import numpy as np

if '/opt/trn_rl_repo' not in sys.path:
    sys.path.insert(0, '/opt/trn_rl_repo')

B, T, E, H, HS = 4, 2048, 512, 8, 64
G = 2          # head groups (cores per batch)
HPG = 4        # heads per group
PAIRS = 2      # head pairs per group
NT = T // 128  # 16 token tiles
NJ = T // 512  # 4 token blocks
NEC = E // 128  # 4 E chunks

_NC_CACHE = {}


def _build_nc():
    import concourse.bacc as bacc
    import concourse.tile as tile
    import concourse.mybir as mybir

    dt = mybir.dt
    F32 = dt.float32
    F32R = dt.float32r
    EXP = mybir.ActivationFunctionType.Exp

    nc = bacc.Bacc("TRN2", target_bir_lowering=False, debug=False, num_devices=8)

    xb = nc.dram_tensor("xb", [T, E], F32, kind="ExternalInput")
    # (proj q/k, pair, echunk) packed: [16, 128, 128] = [k, e_sub, hs2]
    wqk = nc.dram_tensor("wqk", [2 * PAIRS * NEC, 128, 128], F32, kind="ExternalInput")
    wv = nc.dram_tensor("wv", [NEC, 128, HPG * HS], F32, kind="ExternalInput")
    wp = nc.dram_tensor("wp", [PAIRS, 128, E], F32, kind="ExternalInput")
    tri = nc.dram_tensor("tri", [128, 128], F32, kind="ExternalInput")
    ident = nc.dram_tensor("ident", [128, 128], F32, kind="ExternalInput")
    out = nc.dram_tensor("o", [T, E], F32, kind="ExternalOutput")

    with tile.TileContext(nc) as tc:
        import contextlib
        with contextlib.ExitStack() as stack:
            # ---- persistent SBUF ----
            pers = stack.enter_context(tc.tile_pool(name="pers", bufs=1))
            xT = pers.tile([128, NEC, T], F32R, tag="xT")          # [e_sub, ec, t]
            qt = pers.tile([128, PAIRS, T], F32R, tag="qt")        # [2*hs, pair, t]
            kt = pers.tile([128, PAIRS, T], F32R, tag="kt")
            vs = pers.tile([128, NT, HPG * (HS + 1)], F32R, tag="vs")  # [tk_sub, tk_tile, h*65+hs|ones]
            ysb = pers.tile([128, PAIRS, T], F32R, tag="ysb")      # y^T [hh, chunk, t]
            wqk_sb = pers.tile([128, 2 * PAIRS * NEC, 128], F32R, tag="wqk")
            wv_sb = pers.tile([128, NEC, HPG * HS], F32R, tag="wv")
            wp_sb = pers.tile([128, PAIRS, E], F32R, tag="wp")
            tri_sb = pers.tile([128, 128], F32, tag="tri")
            id_sb = pers.tile([128, 128], F32, tag="ident")

            nc.sync.dma_start(wqk_sb[:], wqk.ap().bitcast(F32R).rearrange("k es h -> es k h"))
            nc.sync.dma_start(wv_sb[:], wv.ap().bitcast(F32R).rearrange("c es h -> es c h"))
            nc.sync.dma_start(wp_sb[:], wp.ap().bitcast(F32R).rearrange("c s e -> s c e"))
            nc.sync.dma_start(tri_sb[:], tri[:])
            nc.sync.dma_start(id_sb[:], ident[:])
            # ones columns of v'
            nc.vector.memset(
                vs[:].bitcast(F32).rearrange("p t (h c) -> p t h c", c=HS + 1)[:, :, :, HS:HS + 1],
                1.0)

            # ---- stage A: load x, transpose to xT; stage B: QKV projections ----
            with tc.tile_pool(name="xin", bufs=3) as xin, \
                 tc.tile_pool(name="psA", bufs=2, space="PSUM") as psA, \
                 tc.tile_pool(name="psB", bufs=2, space="PSUM") as psB:
                for tg in range(NJ):  # groups of 4 token tiles
                    xg = xin.tile([128, 4, E], F32, tag="x")
                    nc.sync.dma_start(
                        xg[:], xb.ap().rearrange("(tg tt p) e -> p tg tt e", p=128, tt=4)[:, tg])
                    for ec in range(NEC):
                        ps = psA.tile([128, 512], F32, tag="t")
                        for tt in range(4):
                            nc.tensor.transpose(
                                ps[:, tt * 128:(tt + 1) * 128],
                                xg[:, tt, ec * 128:(ec + 1) * 128], id_sb[:])
                        nc.vector.tensor_copy(xT[:, ec, tg * 512:(tg + 1) * 512], ps[:])

                # q/k projections: out [hs2, t] per (proj, pair)
                for proj in range(2):
                    dst = qt if proj == 0 else kt
                    for p in range(PAIRS):
                        for jj in range(NJ):
                            ps = psB.tile([128, 512], F32, tag="qk")
                            for ec in range(NEC):
                                nc.tensor.matmul(
                                    ps[:],
                                    wqk_sb[:, (proj * PAIRS + p) * NEC + ec, :],
                                    xT[:, ec, jj * 512:(jj + 1) * 512],
                                    start=(ec == 0), stop=(ec == NEC - 1))
                            nc.vector.tensor_copy(dst[:, p, jj * 512:(jj + 1) * 512], ps[:])

                # v projection: out [t_tile, 4*64] natural layout
                for tt in range(NT):
                    ps = psB.tile([128, HPG * HS], F32, tag="v")
                    for ec in range(NEC):
                        nc.tensor.matmul(
                            ps[:], xT[:, ec, tt * 128:(tt + 1) * 128], wv_sb[:, ec, :],
                            start=(ec == 0), stop=(ec == NEC - 1))
                    nc.vector.tensor_copy(
                        vs[:, tt].rearrange("p (h c) -> p h c", c=HS + 1)[:, :, 0:HS],
                        ps[:].rearrange("p (h c) -> p h c", c=HS))

            # ---- stage C: attention + output projection ----
            with tc.tile_pool(name="psS", bufs=1, space="PSUM") as psS, \
                 tc.tile_pool(name="po", bufs=4, space="PSUM") as po, \
                 tc.tile_pool(name="ptp", bufs=3) as ptp, \
                 tc.tile_pool(name="smal", bufs=4) as smal, \
                 tc.tile_pool(name="oout", bufs=3) as oout:
                for j in range(NJ):
                    for p in range(PAIRS):
                        ni = 4 * j + 4  # tk tiles needed
                        outp = [po.tile([HS + 1, 512], F32, tag="o", name=f"outp{h}")
                                for h in range(2)]
                        for i0 in range(0, ni, 2):  # batches of 2 tk tiles
                            sring = psS.tile([128, 4, 512], F32, tag="s")
                            ptile = ptp.tile([128, 4, 512], F32R, tag="pt")
                            nb = min(2, ni - i0)
                            # S^T matmuls: slot = 2*di + h
                            for di in range(nb):
                                i = i0 + di
                                c0 = max(0, 128 * i - 512 * j)
                                for h in range(2):
                                    nc.tensor.matmul(
                                        sring[:, 2 * di + h, c0:512],
                                        kt[h * 64:(h + 1) * 64, p, i * 128:(i + 1) * 128],
                                        qt[h * 64:(h + 1) * 64, p, j * 512 + c0:(j + 1) * 512],
                                        start=True, stop=True)
                            # exp over the whole batch (garbage cols never consumed)
                            cexp = max(0, 128 * i0 - 512 * j)
                            nc.scalar.activation(
                                ptile[:].rearrange("p s c -> p (s c)")[:, cexp:nb * 2 * 512],
                                sring[:].rearrange("p s c -> p (s c)")[:, cexp:nb * 2 * 512],
                                EXP, scale=0.125)
                            for di in range(nb):
                                i = i0 + di
                                off = 128 * i - 512 * j
                                c0 = max(0, off)
                                for h in range(2):
                                    hg = 2 * p + h
                                    if off >= 0:  # diagonal tile: mask strict-lower part
                                        nc.vector.tensor_mul(
                                            ptile[:, 2 * di + h, off:off + 128],
                                            ptile[:, 2 * di + h, off:off + 128],
                                            tri_sb[:])
                                    nc.tensor.matmul(
                                        outp[h][:, c0:512],
                                        vs[:, i, hg * (HS + 1):(hg + 1) * (HS + 1)],
                                        ptile[:, 2 * di + h, c0:512],
                                        start=(i == 0), stop=(i == ni - 1))
                        # normalize: y^T = y'^T / l
                        for h in range(2):
                            r = smal.tile([1, 512], F32, tag="r")
                            nc.vector.reciprocal(r[:], outp[h][HS:HS + 1, :])
                            rb = smal.tile([64, 512], F32, tag="rb")
                            nc.gpsimd.partition_broadcast(rb[:], r[:])
                            nc.vector.tensor_mul(
                                ysb[h * 64:(h + 1) * 64, p, j * 512:(j + 1) * 512],
                                outp[h][0:HS, :], rb[:])
                    # output projection for block j
                    for tt in range(4):
                        ps = po.tile([128, 512], F32, tag="o")
                        for c in range(PAIRS):
                            nc.tensor.matmul(
                                ps[:],
                                ysb[:, c, j * 512 + tt * 128:j * 512 + (tt + 1) * 128],
                                wp_sb[:, c, :],
                                start=(c == 0), stop=(c == PAIRS - 1))
                        os_ = oout.tile([128, 512], F32, tag="os")
                        nc.vector.tensor_copy(os_[:], ps[:])
                        nc.sync.dma_start(out[j * 512 + tt * 128:j * 512 + (tt + 1) * 128, :], os_[:])

    nc.finalize()
    return nc


def _prep_core_inputs(x, wq, wk, wv, wp, b, g):
    xb = np.ascontiguousarray(x[b])
    # wqk pack: [proj, pair, ec, e_sub, hs2]
    wqk = np.empty((2, PAIRS, NEC, 128, 128), np.float32)
    for proj, w in enumerate((wq, wk)):
        for p in range(PAIRS):
            pair = np.concatenate(
                [w[g * HPG + 2 * p], w[g * HPG + 2 * p + 1]], axis=1)  # [512, 128]
            wqk[proj, p] = pair.reshape(NEC, 128, 128)
    wvp = np.concatenate([wv[g * HPG + k] for k in range(HPG)], axis=1)  # [512, 256]
    wvp = wvp.reshape(NEC, 128, HPG * HS)
    wpp = np.ascontiguousarray(wp[g * HPG * HS:(g + 1) * HPG * HS, :]).reshape(PAIRS, 128, E)
    tri = np.triu(np.ones((128, 128), np.float32))
    ident = np.eye(128, dtype=np.float32)
    return dict(
        xb=xb,
        wqk=np.ascontiguousarray(wqk.reshape(2 * PAIRS * NEC, 128, 128)),
        wv=np.ascontiguousarray(wvp),
        wp=wpp,
        tri=tri,
        ident=ident,
    )


def kernel(x, wq, wk, wv, wp, _trace=False, _trace_kwargs=None):
    from concourse.bass_utils import run_bass_kernel_spmd

    if "nc" not in _NC_CACHE:
        _NC_CACHE["nc"] = _build_nc()
    nc = _NC_CACHE["nc"]

    x = np.asarray(x, dtype=np.float32)
    in_maps = [
        _prep_core_inputs(x, wq, wk, wv, wp, c // G, c % G) for c in range(8)
    ]
    kw = {}
    if _trace:
        kw = dict(trace=True, trace_kwargs=_trace_kwargs or {})
    res = run_bass_kernel_spmd(nc, in_maps, core_ids=list(range(8)), **kw)
    out = np.empty((B, T, E), np.float32)
    for b in range(B):
        out[b] = res.results[2 * b]["o"] + res.results[2 * b + 1]["o"]
    if _trace:
        return out, res
    return out
